# revision 14
# baseline (speedup 1.0000x reference)
"""DecoderAttention (GQA + RoPE + causal) Trainium2 Bass kernel.

Sharding over 8 NeuronCores: core = 4*batch + g where g in [0,4) is the
head-group. Each core computes 4 query heads (o-slice 512g:512g+512 of Wq)
and their shared KV head (slice 128g:128g+128 of Wk/Wv), plus the partial
output projection with the matching 512-column slice of Wo. Host sums the 4
partials per batch.

Per-core dataflow (matmul inputs in bf16 by default, f32 PSUM accumulate):
  QT[o,t] = WqT.T @ hsT   (transposed projections; hsT streamed once)
  RoPE applied in [d,t] layout via partition-offset DVE ops
  ST[k,q] = KT_tile.T @ QT  -> exp on ACT (scale folded) -> P[k,q]
  P tiles accumulated into P_acc on vector; rowsum = ones.T @ P_acc (one
  matmul pair per head-pair/macro instead of per-visit rowsum matmuls)
  attn_outT[d,q] += V_tile.T @ P (sliced causally)
  normalize at the PSUM->SBUF copy; out[t,h] += ao_tile.T @ WoT, drained
  to bf16 on the scalar engine and DMA'd per 2048-wide row block.
"""
import math
import os
import sys

sys.path.insert(0, "/opt/trn_rl_repo")

import numpy as np
import ml_dtypes

import concourse.bass as bass  # noqa: F401  (registers engines)
import concourse.mybir as mybir
import concourse.tile as tile
from concourse import bacc
from concourse.bass_utils import run_bass_kernel_spmd

B, T, HID = 2, 2048, 2048
H, KVH, D = 16, 4, 128
NH = H // KVH          # q-heads per core = 4
TM = 512               # t/q macro tile
NKT = HID // 128       # 16 contraction k-tiles for projections
NTT = T // 128         # 16 t-tiles
NM = T // TM           # 4 macros
SCALE = 1.0 / math.sqrt(D)
NEG = -1.0e30

f32 = mybir.dt.float32
f32r = mybir.dt.float32r
bf16 = mybir.dt.bfloat16
# matmul-input dtype: "bf16" (fast, ~3e-3 rel err) or "f32r" (2x slower, ~2e-4)
MMDT_NAME = os.environ.get("KERNEL_MMDT", "bf16")
MMDT = {"bf16": bf16, "f32r": f32r}[MMDT_NAME]
NP_IN = ml_dtypes.bfloat16 if MMDT_NAME == "bf16" else np.float32
DRAM_IN = bf16 if MMDT_NAME == "bf16" else f32
FAST = MMDT_NAME == "bf16"   # vector-accumulated rowsum + bf16 out path
EXP = mybir.ActivationFunctionType.Exp
IDENT = mybir.ActivationFunctionType.Identity
MULT = mybir.AluOpType.mult
ADD = mybir.AluOpType.add

OUT_DT = bf16 if FAST else f32
NP_OUT = ml_dtypes.bfloat16 if FAST else np.float32

LAST_RESULTS = None  # BassKernelResults of the most recent run (for test.py)

_cache = {}


def _ldin(nc, dst, src_ap):
    """DMA a matmul input: plain HWDGE when dtypes match, gpsimd cast DMA
    otherwise (f32 -> f32r)."""
    if DRAM_IN == bf16:
        nc.sync.dma_start(out=dst, in_=src_ap)
    else:
        nc.gpsimd.dma_start(out=dst, in_=src_ap)


def _emit(nc, tc, causal):
    ap = {}
    ap["hsT"] = nc.dram_tensor("hsT", [HID, T], DRAM_IN, kind="ExternalInput").ap()
    ap["wqkvT"] = nc.dram_tensor("wqkvT", [HID, 768], DRAM_IN, kind="ExternalInput").ap()
    ap["woT"] = nc.dram_tensor("woT", [512, HID], DRAM_IN, kind="ExternalInput").ap()
    ap["bias"] = nc.dram_tensor("bias", [128, 6], f32, kind="ExternalInput").ap()
    ap["cosT"] = nc.dram_tensor("cosT", [D, T], f32, kind="ExternalInput").ap()
    ap["sinTs"] = nc.dram_tensor("sinTs", [D, T], f32, kind="ExternalInput").ap()
    ap["dmask"] = nc.dram_tensor("dmask", [128, 128], DRAM_IN, kind="ExternalInput").ap()
    ap["ones1"] = nc.dram_tensor("ones1", [128, 1], DRAM_IN, kind="ExternalInput").ap()
    ap["ident"] = nc.dram_tensor("ident", [128, 128], f32, kind="ExternalInput").ap()
    if not causal:
        ap["maskT"] = nc.dram_tensor("maskT", [T, T], f32, kind="ExternalInput").ap()
    out_part = nc.dram_tensor("out_part", [T, HID], OUT_DT, kind="ExternalOutput").ap()

    # grouped views: 4 contraction k-tiles per DMA
    hsT_g = ap["hsT"].rearrange("(g p) t -> p g t", p=128)      # [128, 16, T]
    wqkv_g = ap["wqkvT"].rearrange("(g p) f -> p g f", p=128)   # [128, 16, 768]
    wo_g = ap["woT"].rearrange("(o p) f -> p o f", p=128)       # [128, 4, HID]

    with tc.tile_pool(name="persist", bufs=1) as pper, \
         tc.tile_pool(name="wqkv", bufs=1) as pw, \
         tc.tile_pool(name="wo", bufs=1) as pwo, \
         tc.tile_pool(name="ropecs", bufs=1) as pcs, \
         tc.tile_pool(name="phA", bufs=2) as pa, \
         tc.tile_pool(name="hst", bufs=8) as ph, \
         tc.tile_pool(name="ptile", bufs=5) as pp, \
         tc.tile_pool(name="pacc", bufs=2) as pac, \
         tc.tile_pool(name="phB", bufs=2) as pb, \
         tc.tile_pool(name="mask", bufs=3) as pm, \
         tc.tile_pool(name="outp", bufs=2) as po:
        qt = [pper.tile([128, T], MMDT, tag=f"qt{h}", name=f"qt{h}") for h in range(NH)]
        kt = pper.tile([128, T], MMDT, tag="kt", name="kt")
        vsb = pper.tile([128, T], MMDT, tag="vsb", name="vsb")
        ao = [pper.tile([128, T], MMDT, tag=f"ao{h}", name=f"ao{h}") for h in range(NH)]

        # ---- DMA issue: two queues in parallel (sync + gpsimd). ----
        # gpsimd queue: bias first (needed at first PSUM drain), then macro-0
        # h-groups, then cos/sin + small constants.
        # sync queue: weight groups, wo, macro-1 h-groups; later macros
        # streamed inside the loop.
        bias_t = pper.tile([128, 6], f32, tag="bias", name="bias")

        h_groups = {}   # (macro, group) -> tile [128, 4, 512] (FAST mode)
        h_single = {}   # (macro, k) -> tile [128, 512]

        def issue_h(m, engine, singles=False):
            tsl = slice(TM * m, TM * (m + 1))
            if not FAST:
                for k in range(NKT):
                    h_t = ph.tile([128, TM], MMDT, tag="hst1", name="hst1")
                    nc.gpsimd.dma_start(
                        out=h_t[:], in_=ap["hsT"][128 * k:128 * (k + 1), tsl])
                    h_single[(m, k)] = h_t
                return
            g0 = 0
            if singles:
                # first k-tiles as small single loads so the PE can start
                # as early as possible (DMA rings round-robin: smaller
                # first-needed transfers complete sooner)
                for k in range(4):
                    h_t = ph.tile([128, TM], MMDT, tag="hsts", name="hsts")
                    engine.dma_start(
                        out=h_t[:], in_=ap["hsT"][128 * k:128 * (k + 1), tsl])
                    h_single[(m, k)] = h_t
                g0 = 1
            for g in range(g0, 4):
                hg = ph.tile([128, 4, TM], MMDT, tag="hst", name="hst")
                engine.dma_start(
                    out=hg[:], in_=hsT_g[:, 4 * g:4 * (g + 1), tsl])
                h_groups[(m, g)] = hg

        def h_ap(m, k):
            if (m, k) in h_single:
                return h_single[(m, k)][:]
            return h_groups[(m, k // 4)][:, k % 4, :]

        w_tiles = {}
        if FAST:
            # consumption-ordered, split across the sync (w) and gpsimd (h)
            # queues; first four of each as singles for fast arrival.
            for k in range(4):
                wt = pw.tile([128, 768], MMDT, tag=f"ws{k}", name=f"ws{k}")
                nc.sync.dma_start(
                    out=wt[:], in_=ap["wqkvT"][128 * k:128 * (k + 1), :])
                w_tiles[("s", k)] = wt
            issue_h(0, nc.gpsimd, singles=True)
            for g in range(1, 4):
                wt = pw.tile([128, 4, 768], MMDT, tag=f"w{g}", name=f"w{g}")
                nc.sync.dma_start(out=wt[:], in_=wqkv_g[:, 4 * g:4 * (g + 1), :])
                w_tiles[("g", g)] = wt
        else:
            for k in range(NKT):
                wt = pw.tile([128, 768], MMDT, tag=f"ws{k}", name=f"ws{k}")
                nc.gpsimd.dma_start(
                    out=wt[:], in_=ap["wqkvT"][128 * k:128 * (k + 1), :])
                w_tiles[("s", k)] = wt
            issue_h(0, nc.gpsimd)

        def w_ap(k, csl):
            if ("s", k) in w_tiles:
                return w_tiles[("s", k)][:, csl]
            return w_tiles[("g", k // 4)][:, k % 4, csl]

        nc.gpsimd.dma_start(out=bias_t[:], in_=ap["bias"][:])
        cos_t = pcs.tile([128, T], f32, tag="cosT", name="cosT")
        sins_t = pcs.tile([128, T], f32, tag="sinTs", name="sinTs")
        # rope tables per-macro so the first macro's slice arrives early
        nc.gpsimd.dma_start(out=cos_t[:, 0:TM], in_=ap["cosT"][:, 0:TM])
        nc.gpsimd.dma_start(out=sins_t[:, 0:TM], in_=ap["sinTs"][:, 0:TM])
        ident_t = pper.tile([128, 128], f32, tag="ident", name="ident")
        nc.gpsimd.dma_start(out=ident_t[:], in_=ap["ident"][:])

        wo_t = pwo.tile([128, 4, HID], MMDT, tag="wo", name="wo")
        if FAST:
            issue_h(1, nc.sync)
            nc.sync.dma_start(out=wo_t[:], in_=wo_g[:])
        else:
            issue_h(1, nc.gpsimd)
            nc.gpsimd.dma_start(out=wo_t[:], in_=wo_g[:])

        nc.gpsimd.dma_start(out=cos_t[:, TM:], in_=ap["cosT"][:, TM:])
        nc.gpsimd.dma_start(out=sins_t[:, TM:], in_=ap["sinTs"][:, TM:])
        ones_r = pper.tile([128, 1], MMDT, tag="ones1", name="ones1")
        _ldin(nc, ones_r[:], ap["ones1"][:])
        dmask_t = pper.tile([128, 128], MMDT, tag="dmask", name="dmask")
        _ldin(nc, dmask_t[:], ap["dmask"][:])

        # ---------------- Phase A: projections + RoPE + V transpose ---------
        with tc.tile_pool(name="psA", bufs=1, space="PSUM") as psa, \
             tc.tile_pool(name="psAtr", bufs=2, space="PSUM") as psatr:

            def rope(dst, src, tsl):
                # dst = src*cos + rotate_half(src)*sin, in [d, t] layout.
                # sins_t rows d<64 hold +sin[d+64], rows d>=64 hold -sin[d-64],
                # so each mul reads both SBUF inputs at the same base partition
                # (walrus requires equal input base partitions); only the
                # output is partition-shifted.
                tmp = pa.tile([128, TM], f32, tag="ropetmp", name="ropetmp")
                nc.vector.tensor_tensor(
                    out=tmp[0:64, :], in0=src[64:128, :], in1=sins_t[64:128, tsl], op=MULT)
                nc.vector.tensor_tensor(
                    out=tmp[64:128, :], in0=src[0:64, :], in1=sins_t[0:64, tsl], op=MULT)
                tmp2 = pa.tile([128, TM], f32, tag="ropetmp2", name="ropetmp2")
                nc.gpsimd.tensor_tensor(
                    out=tmp2[:], in0=src[:], in1=cos_t[:, tsl], op=MULT)
                nc.gpsimd.tensor_tensor(out=dst, in0=tmp2[:], in1=tmp[:], op=ADD)

            vraw_prev = None

            def v_transposes(m, vraw):
                for j in range(4):
                    tt = 4 * m + j
                    tr_ps = psatr.tile([128, 128], f32, tag="vtr", name="vtr")
                    nc.tensor.transpose(
                        tr_ps[:], vraw[:, 128 * j:128 * (j + 1)], ident_t[:])
                    nc.scalar.copy(vsb[:, 128 * tt:128 * (tt + 1)], tr_ps[:])

            for m in range(NM):
                if m + 2 < NM:
                    issue_h(m + 2, nc.sync if FAST else nc.gpsimd)
                tsl = slice(TM * m, TM * (m + 1))
                q_ps = [psa.tile([128, TM], f32, tag=f"psq{o}", name=f"psq{o}")
                        for o in range(NH)]
                k_ps = psa.tile([128, TM], f32, tag="psk", name="psk")
                v_ps = psa.tile([128, TM], f32, tag="psv", name="psv")
                for k in range(NKT):
                    h_t = h_ap(m, k)
                    st = (k == 0)
                    sp = (k == NKT - 1)
                    for o in range(NH):
                        nc.tensor.matmul(
                            q_ps[o][:], w_ap(k, slice(128 * o, 128 * (o + 1))),
                            h_t, start=st, stop=sp)
                    nc.tensor.matmul(
                        k_ps[:], w_ap(k, slice(512, 640)), h_t, start=st, stop=sp)
                    nc.tensor.matmul(
                        v_ps[:], w_ap(k, slice(640, 768)), h_t, start=st, stop=sp)
                # previous macro's V transposes: their inputs are long since
                # ready, so they never stall the PE at the macro boundary.
                if vraw_prev is not None:
                    v_transposes(m - 1, vraw_prev)
                # drain the six accumulators on two engines in parallel so
                # the next macro's matmuls get their PSUM banks back quickly
                raws = []
                for o in range(NH):
                    qraw = pa.tile([128, TM], f32, tag=f"qraw{o}", name=f"qraw{o}")
                    if o % 2 == 0 or m == NM - 1:
                        nc.scalar.activation(
                            qraw[:], q_ps[o][:], IDENT, bias=bias_t[:, o:o + 1])
                    else:
                        nc.vector.tensor_scalar_add(
                            qraw[:], q_ps[o][:], bias_t[:, o:o + 1])
                    raws.append(qraw)
                kraw = pa.tile([128, TM], f32, tag="kraw", name="kraw")
                nc.scalar.activation(kraw[:], k_ps[:], IDENT, bias=bias_t[:, 4:5])
                for o in range(NH):
                    rope(qt[o][:, tsl], raws[o], tsl)
                rope(kt[:, tsl], kraw, tsl)
                vraw = pa.tile([128, TM], f32, tag="vraw", name="vraw", bufs=2)
                nc.scalar.activation(vraw[:], v_ps[:], IDENT, bias=bias_t[:, 5:6])
                vraw_prev = vraw
            v_transposes(NM - 1, vraw_prev)

        # ---------------- Phase B + C: attention + output projection --------
        # Phase C (output projection) of each macro is interleaved into the
        # NEXT macro's visit loop so its matmuls fill the exp-paced gaps on
        # the PE instead of running as a serial block.
        with tc.tile_pool(name="psSC", bufs=2, space="PSUM") as ps_sc, \
             tc.tile_pool(name="psAV", bufs=1, space="PSUM") as ps_av, \
             tc.tile_pool(name="psRS", bufs=1, space="PSUM") as ps_rs:
            c_pending = []   # (tt, hc) output-projection units of prev macro
            c_state = {}

            def emit_c_unit():
                if not c_pending:
                    return
                tt, hc = c_pending.pop(0)
                ttsl = slice(128 * tt, 128 * (tt + 1))
                if hc == 0:
                    c_state[tt] = po.tile([128, HID], OUT_DT, tag="ot", name="ot")
                ot = c_state[tt]
                hsl = slice(512 * hc, 512 * (hc + 1))
                op_ps = ps_rs.tile([128, TM], f32, tag=f"rs{hc % 2}", name="opps")
                for o in range(4):
                    nc.tensor.matmul(
                        op_ps[:], ao[o][:, ttsl], wo_t[:, o, hsl],
                        start=(o == 0), stop=(o == 3))
                nc.scalar.activation(ot[:, hsl], op_ps[:], IDENT)
                if hc == 3:
                    nc.sync.dma_start(out=out_part[ttsl, :], in_=ot[:])

            for m in (1, 2, 3, 0):
                nk = 4 * (m + 1) if causal else NTT
                crate = 2 if (m == 0 and causal) else 1
                qsl = slice(TM * m, TM * (m + 1))
                for pair in range(NH // 2):
                    h0, h1 = 2 * pair, 2 * pair + 1
                    av = [ps_av.tile([128, TM], f32, tag=f"av{i}", name=f"av{i}")
                          for i in range(2)]
                    if FAST:
                        pacc_v = pac.tile([128, 2 * TM], MMDT, tag="paccv",
                                          name="paccv")
                        use_g = nk >= 8
                        if use_g:
                            pacc_g = pac.tile([128, 2 * TM], MMDT, tag="paccg",
                                              name="paccg")
                    else:
                        rs = [ps_rs.tile([1, TM], f32, tag=f"rs{i}", name=f"rs{i}")
                              for i in range(2)]
                    pt_prev = None

                    def _q0(kk):
                        # first q column this visit contributes to (causal):
                        # q_local < 128*jp is entirely masked, never read
                        jp = kk - (nk - 4)
                        return 128 * jp if (causal and jp > 0) else 0

                    def av_mms(kk, pt):
                        q0 = _q0(kk)
                        ksl = slice(128 * kk, 128 * (kk + 1))
                        st = (kk == 0)
                        sp = (kk == nk - 1)
                        nc.tensor.matmul(av[0][:, q0:TM], vsb[:, ksl],
                                         pt[:, q0:TM], start=st, stop=sp)
                        nc.tensor.matmul(av[1][:, q0:TM], vsb[:, ksl],
                                         pt[:, TM + q0:2 * TM], start=st, stop=sp)

                    def rs_mms(kk, pt):
                        q0 = _q0(kk)
                        st = (kk == 0)
                        sp = (kk == nk - 1)
                        nc.tensor.matmul(rs[0][:, q0:TM], ones_r[:],
                                         pt[:, q0:TM], start=st, stop=sp)
                        nc.tensor.matmul(rs[1][:, q0:TM], ones_r[:],
                                         pt[:, TM + q0:2 * TM], start=st, stop=sp)

                    for kk in range(nk):
                        ksl = slice(128 * kk, 128 * (kk + 1))
                        q0 = _q0(kk)
                        # both heads' score tiles side by side -> one exp pass
                        sc = ps_sc.tile([128, 2 * TM], f32, tag="sc", name="sc")
                        nc.tensor.matmul(sc[:, q0:TM], kt[:, ksl],
                                         qt[h0][:, qsl][:, q0:],
                                         start=True, stop=True)
                        nc.tensor.matmul(sc[:, TM + q0:2 * TM], kt[:, ksl],
                                         qt[h1][:, qsl][:, q0:],
                                         start=True, stop=True)
                        pt = pp.tile([128, 2 * TM], MMDT, tag="pt", name="pt")
                        if causal:
                            if q0 == 0:
                                nc.scalar.activation(pt[:], sc[:], EXP, scale=SCALE)
                            else:
                                nc.scalar.activation(
                                    pt[:, q0:TM], sc[:, q0:TM], EXP, scale=SCALE)
                                nc.scalar.activation(
                                    pt[:, TM + q0:2 * TM], sc[:, TM + q0:2 * TM],
                                    EXP, scale=SCALE)
                            jp = kk - (nk - 4)
                            if jp >= 0:
                                # mask after exp: only the 128x128 diagonal
                                # triangle is ever read partially masked (the
                                # region left of it is skipped by the sliced
                                # AV/rowsum/accumulate ops)
                                w0, w1 = 128 * jp, 128 * (jp + 1)
                                for base in (0, TM):
                                    nc.vector.tensor_tensor(
                                        out=pt[:, base + w0:base + w1],
                                        in0=pt[:, base + w0:base + w1],
                                        in1=dmask_t[:], op=MULT)
                        else:
                            mk = pm.tile([128, TM], f32, tag="mk", name="mk")
                            nc.sync.dma_start(
                                out=mk[:], in_=ap["maskT"][ksl, qsl])
                            for base in (0, TM):
                                nc.vector.scalar_tensor_tensor(
                                    out=sc[:, base:base + TM],
                                    in0=sc[:, base:base + TM],
                                    scalar=SCALE, in1=mk[:],
                                    op0=MULT, op1=ADD)
                            nc.scalar.activation(pt[:], sc[:], EXP, scale=1.0)
                        if FAST:
                            # accumulate P for the rowsum: split across the
                            # vector and gpsimd engines (both are far slower
                            # than their specs for tensor_tensor, so neither
                            # can absorb the whole stream)
                            on_g = use_g and kk % 3 == 2
                            eng = nc.gpsimd if on_g else nc.vector
                            tgt = pacc_g if on_g else pacc_v
                            first = (kk == 0) or (on_g and kk == 2)
                            if first:
                                eng.tensor_copy(tgt[:], pt[:])
                            elif q0 == 0:
                                eng.tensor_tensor(
                                    out=tgt[:], in0=tgt[:], in1=pt[:], op=ADD)
                            else:
                                for base in (0, TM):
                                    eng.tensor_tensor(
                                        out=tgt[:, base + q0:base + TM],
                                        in0=tgt[:, base + q0:base + TM],
                                        in1=pt[:, base + q0:base + TM], op=ADD)
                        else:
                            rs_mms(kk, pt)
                        # software pipeline: AV of the previous visit fills the
                        # PE while this visit's exp runs.
                        if pt_prev is not None:
                            av_mms(kk - 1, pt_prev)
                        pt_prev = pt
                        if FAST:
                            for _ in range(crate):
                                emit_c_unit()
                    av_mms(nk - 1, pt_prev)
                    if FAST:
                        rs = [ps_rs.tile([1, TM], f32, tag=f"rs{i}", name=f"rs{i}")
                              for i in range(2)]
                        # rowsum matmuls over the accumulated P (both halves)
                        for i, base in ((0, 0), (1, TM)):
                            nc.tensor.matmul(
                                rs[i][:], ones_r[:], pacc_v[:, base:base + TM],
                                start=True, stop=not use_g)
                            if use_g:
                                nc.tensor.matmul(
                                    rs[i][:], ones_r[:], pacc_g[:, base:base + TM],
                                    start=False, stop=True)
                    for i in range(2):
                        inv = pb.tile([1, TM], f32, tag="inv", name="inv")
                        nc.vector.reciprocal_approx_fast(out=inv[:], in_=rs[i][:])
                        invb = pb.tile([128, TM], f32, tag=f"invb{i}",
                                       name=f"invb{i}")
                        nc.gpsimd.partition_broadcast(invb[:], inv[:])
                        if i == 0:
                            invb0 = invb
                        else:
                            invb1 = invb
                    for i, h, invb in ((0, h0, invb0), (1, h1, invb1)):
                        nc.vector.tensor_tensor(
                            out=ao[h][:, qsl], in0=av[i][:], in1=invb[:], op=MULT)
                # register this macro's output-projection units; they are
                # interleaved into the next macro's visit loop (FAST), or
                # emitted serially here (fallback).
                while c_pending:
                    emit_c_unit()
                c_pending = [(4 * m + j, hc) for j in range(4) for hc in range(4)]
                if not FAST:
                    while c_pending:
                        emit_c_unit()
            while c_pending:
                emit_c_unit()


def _build(causal):
    nc = bacc.Bacc("TRN2", target_bir_lowering=False, debug=False, num_devices=8)
    with tile.TileContext(nc) as tc:
        _emit(nc, tc, causal)
    nc.compile()
    return nc


def _canonical_causal_mask():
    neg = np.float32(np.finfo(np.float32).min)
    m = np.where(np.tril(np.ones((T, T), dtype=bool)), np.float32(0.0), neg)
    return m.astype(np.float32)


def kernel(**inputs):
    global LAST_RESULTS
    hs = np.ascontiguousarray(np.asarray(inputs["hidden_states"], dtype=np.float32))
    cos = np.asarray(inputs["cos"], dtype=np.float32)
    sin = np.asarray(inputs["sin"], dtype=np.float32)
    mask = np.asarray(inputs["attention_mask"], dtype=np.float32)
    Wq = np.asarray(inputs["Wq"], dtype=np.float32)
    Wk = np.asarray(inputs["Wk"], dtype=np.float32)
    Wv = np.asarray(inputs["Wv"], dtype=np.float32)
    Wo = np.asarray(inputs["Wo"], dtype=np.float32)
    bq = np.asarray(inputs["bq"], dtype=np.float32)
    bk = np.asarray(inputs["bk"], dtype=np.float32)
    bv = np.asarray(inputs["bv"], dtype=np.float32)

    causal = bool(np.array_equal(mask[0, 0], _canonical_causal_mask()))

    key = (causal, MMDT_NAME)
    if key not in _cache:
        _cache[key] = _build(causal)
    nc = _cache[key]

    tri01 = (np.arange(128)[:, None] <= np.arange(128)[None, :])
    dmask = tri01.astype(NP_IN)
    ident = np.eye(128, dtype=np.float32)
    ones1 = np.ones((128, 1), dtype=NP_IN)
    if not causal:
        maskT = np.ascontiguousarray(mask[0, 0].T)

    in_maps = []
    for c in range(8):
        b, g = divmod(c, 4)
        sl_q = slice(512 * g, 512 * (g + 1))
        sl_kv = slice(128 * g, 128 * (g + 1))
        sinT = np.ascontiguousarray(sin[b].T)  # [D, T]
        # row d<64: +sin[d+64] (consumed at base partition 0 writing rows 64:128)
        # row d>=64: -sin[d-64] (consumed at base partition 64 writing rows 0:64)
        sinTs = np.concatenate([sinT[64:128], -sinT[0:64]], axis=0)
        bias = np.zeros((128, 6), dtype=np.float32)
        bias[:, 0:4] = bq[sl_q].reshape(4, 128).T
        bias[:, 4] = bk[sl_kv]
        bias[:, 5] = bv[sl_kv]
        m = {
            "hsT": np.ascontiguousarray(hs[b].T.astype(NP_IN)),
            "wqkvT": np.ascontiguousarray(
                np.concatenate([Wq[sl_q], Wk[sl_kv], Wv[sl_kv]], axis=0).T.astype(NP_IN)),
            "woT": np.ascontiguousarray(Wo[:, sl_q].T.astype(NP_IN)),
            "bias": bias,
            "cosT": np.ascontiguousarray(cos[b].T),
            "sinTs": np.ascontiguousarray(sinTs),
            "dmask": dmask,
            "ones1": ones1,
            "ident": ident,
        }
        if not causal:
            m["maskT"] = maskT
        in_maps.append(m)

    trace = os.environ.get("KERNEL_TRACE", "0") == "1"
    res = run_bass_kernel_spmd(nc, in_maps, list(range(8)), trace=trace)
    LAST_RESULTS = res

    out = np.empty((B, T, HID), dtype=np.float32)
    for b in range(B):
        acc = res.results[4 * b]["out_part"].astype(np.float32)
        for g in range(1, 4):
            acc += res.results[4 * b + g]["out_part"].astype(np.float32)
        out[b] = acc
    return out
